# revision 28
# baseline (speedup 1.0000x reference)
"""Bass/Trainium2 kernel for the (dead-attention) GAT reference.

Effective math (see reference):
    h1  = x @ W1f                 W1f = W1.transpose(1,0,2).reshape(256,128)
    hp1 = elu(adj @ h1)
    h2  = hp1 @ W2f               W2f = W2.transpose(1,0,2).reshape(128,128)
    hp2 = elu(adj @ h2)
    y   = elu(hp2 @ Wout + bout)
    out = log_softmax(y, axis=1)

v3 design:
  * adj stored as fp8 e4m3 residual  R16 = 16*(adj - 0.5)  and h1/h2
    stationaries in e4m3, so every big matmul runs in DoubleRow perf
    mode: one instruction contracts a PAIR of 128-row k-blocks at 0.5
    cycles/row.  This halves HBM traffic (vs fp16) and roughly halves
    PE instruction count/stream time.  The rank-1 term 0.5*J@h lost by
    the residual encoding is restored per feature when reading PSUM:
        hp = elu(psum/16 + 0.5*colsum(h))
    corr1 = 0.5*colsum(x)@W1f is a host-prepared constant; corr2 is
    assembled from per-core colsum(x2) partials carried in the gather
    payloads as e4m3 hi/lo pairs, then one on-device fp32 matvec.
    Verified numerics vs fp32 reference (host emulation): 3.1e-4.
  * adj rows (contraction dim) use a GLOBAL order (all ranks' first-
    half nodes, then all ranks' second halves), identical on every
    core.  Layer 1 runs as two column-half passes; each pass's h2 half
    is AllGathered while subsequent compute streams, hiding the
    collectives.  Payloads are partition-major (one DMA line per
    partition) and gathered blocks DMA straight into the node-major
    stationary buffer hs2 with one plain 2D copy per rank - no PE
    transposes, no dynamic offsets.
  * Layer-1 passes stream their adj groups in REVERSED order so the
    last `adj_bufs` group tiles of both column halves stay resident in
    SBUF and layer 2 re-reads that much less adj from HBM.
"""

import sys

import numpy as np

sys.path.insert(0, "/opt/trn_rl_repo")

N = 16384  # nodes
F = 256  # input features
D = 128  # hidden width (nheads*nhid)
C = 32  # classes
NCORES = 8
S = N // NCORES  # rows per core

_nc_cache = {}


def build_gat_nc(n_total=N, ncores=NCORES, enable_asserts=False, adj_bufs=8, kg=8):
    """Build the SPMD Bass program (one program, runs on all cores)."""
    from concourse import bacc, mybir, tile

    s = n_total // ncores  # own rows per core
    half = s // 2  # column-half width
    kb = n_total // 128  # contraction blocks
    ngrp = kb // kg  # adj DMA groups
    kb8 = kb // 8  # xc chunk groups
    c8 = half // 128  # 128-node blocks per (rank, half)
    rc = s // 128  # 128-row chunks of the own shard
    f32 = mybir.dt.float32
    f16 = mybir.dt.float16
    f8 = mybir.dt.float8e4
    AF = mybir.ActivationFunctionType
    OP = mybir.AluOpType
    DR = mybir.MatmulPerfMode.DoubleRow

    def chunks(width):
        return [(o, min(512, width - o)) for o in range(0, width, 512)]

    nc = bacc.Bacc(
        "TRN2",
        target_bir_lowering=False,
        debug=False,
        enable_asserts=enable_asserts,
        num_devices=ncores,
    )

    adjt0 = nc.dram_tensor("adjt0", [n_total, half], f8, kind="ExternalInput")
    adjt1 = nc.dram_tensor("adjt1", [n_total, half], f8, kind="ExternalInput")
    xc = nc.dram_tensor("xc", [kb8 * 128, 8 * F], f8, kind="ExternalInput")
    w1 = nc.dram_tensor("w1", [F, D], f8, kind="ExternalInput")
    w2 = nc.dram_tensor("w2", [D, D], f16, kind="ExternalInput")
    wout = nc.dram_tensor("wout", [D, C], f32, kind="ExternalInput")
    bb = nc.dram_tensor("bb", [128, C], f32, kind="ExternalInput")
    corr1 = nc.dram_tensor("corr1", [128, 1], f32, kind="ExternalInput")
    out = nc.dram_tensor("out", [s, C], f32, kind="ExternalOutput")

    rg = [list(range(ncores))]
    pw = c8 * 128 + 2  # payload width: c8 h2 blocks + colsum hi/lo

    with tile.TileContext(nc) as tc:
        with (
            tc.tile_pool(name="dram", bufs=1, space="DRAM") as dram,
            tc.tile_pool(name="const", bufs=1) as const,
            tc.tile_pool(name="hs1p", bufs=1) as hs1p,
            tc.tile_pool(name="hs2p", bufs=1) as hs2p,
            tc.tile_pool(name="ap0", bufs=adj_bufs) as ap0,
            tc.tile_pool(name="ap1", bufs=adj_bufs) as ap1,
            tc.tile_pool(name="xcp", bufs=4) as xcpool,
            tc.tile_pool(name="xe", bufs=2) as xepool,
            tc.tile_pool(name="xh", bufs=2) as xhpool,
            tc.tile_pool(name="h2o", bufs=2) as h2opool,
            tc.tile_pool(name="tmp", bufs=1) as tmp,
            tc.tile_pool(name="outp", bufs=2) as outp,
            tc.tile_pool(name="stat", bufs=1) as stat,
            tc.tile_pool(name="big", bufs=4, space="PSUM") as big,
            tc.tile_pool(name="pss", bufs=2, space="PSUM") as pss,
            tc.tile_pool(name="psy", bufs=2, space="PSUM") as psy,
        ):
            # ring assignment:
            #   A/B (sync + scalar HWDGE): xc and the big adj streams only
            #   G (gpsimd SWDGE): constants, bounces, collective triggers,
            #     gather pulls, output stores.  NOTE: keep collective-output
            #     reads off the sync ring (test_sync_dma_collective_hang).
            ringA, ringB = nc.sync, nc.scalar
            ringG = nc.gpsimd

            # --- replicated constants ---
            w1s = const.tile([128, 2, D], f8, tag="w1s")
            ringG.dma_start(w1s[:], w1.ap().rearrange("(a p) m -> p a m", p=128))
            w2s = const.tile([128, D], f16, tag="w2s")
            ringG.dma_start(w2s[:], w2.ap())
            wouts = const.tile([128, C], f32, tag="wouts")
            ringG.dma_start(wouts[:], wout.ap())
            bbs = const.tile([128, C], f32, tag="bbs")
            ringG.dma_start(bbs[:], bb.ap())
            corr1s = const.tile([128, 1], f32, tag="corr1s")
            ringG.dma_start(corr1s[:], corr1.ap())
            # fp32 copy of W2f for the (fp32) corr2 matvec
            w2s32 = const.tile([128, D], f32, tag="w2s32")
            nc.vector.tensor_copy(w2s32[:], w2s[:])

            # --- tiny warm-up collective: absorbs the CC-stream wakeup
            # latency (~11us) while the stream is otherwise idle ---
            zs = const.tile([1, 64], f8, tag="zs")
            nc.vector.memset(zs[:], 0.0)
            dumb = dram.tile([1, 64], f8, tag="dumb")
            dumf = dram.tile([ncores, 64], f8, tag="dumf", addr_space="Shared")
            ringG.dma_start(dumb[:], zs[:])
            ringG.collective_compute(
                "AllGather", OP.bypass, ins=[dumb.opt()], outs=[dumf.opt()],
                replica_groups=rg,
            )

            # --- DRAM bounce buffers for the collectives (partition-major) ---
            h2bA = dram.tile([128, pw], f8, tag="h2bA")
            h2bB = dram.tile([128, pw], f8, tag="h2bB")
            h2fA = dram.tile([ncores * 128, pw], f8, tag="h2fA", addr_space="Shared")
            h2fB = dram.tile([ncores * 128, pw], f8, tag="h2fB", addr_space="Shared")

            # ---- h1 = x @ W1f, replicated (DoubleRow over the two F-halves) ----
            # xc group g holds 8 chunk-lhsTs contiguous per partition:
            # xc[g*128+p, ((j*2+a)*128)+m] = xperm.T[a*128+p, (g*8+j)*128+m]
            hs1 = hs1p.tile([128, kb, D], f8, tag="hs1")
            xr = xc.ap().rearrange("(g p) q -> g p q", p=128)
            xg = None
            for k4 in range(kb // 4):
                ph = big.tile([128, 4, D], f32, tag="big", name=f"ph1_{k4}")
                for i in range(4):
                    k = k4 * 4 + i
                    g, j = divmod(k, 8)
                    if j == 0:
                        xg = xcpool.tile([128, 8, 2, 128], f8, tag="xg")
                        (ringA if g % 2 == 0 else ringB).dma_start(
                            xg.rearrange("p j a m -> p (j a m)"), xr[g]
                        )
                    nc.tensor.matmul(
                        ph[:, i, :], xg[:, j, :, :], w1s[:],
                        start=(i == 0), stop=(i == 3), perf_mode=DR,
                    )
                nc.vector.tensor_copy(hs1[:, k4 * 4 : (k4 + 1) * 4, :], ph[:])

            ar0 = adjt0.ap().rearrange("(g p j) m -> g p (j m)", p=128, j=kg)
            ar1 = adjt1.ap().rearrange("(g p j) m -> g p (j m)", p=128, j=kg)

            def stream_pass(ar, apool, nch, inject):
                """One reversed-order DoubleRow streaming pass over `ar`.

                nch: list of (psum_tile, o, w) output chunks.
                inject: {group_iteration_index: callback} runs after that
                group's matmuls are emitted.
                Returns {g: sbuf tile}; entries g < adj_bufs stay resident.
                """
                tiles = {}
                kgp = kg // 2
                for gi, g in enumerate(reversed(range(ngrp))):
                    at = apool.tile([128, kg * half], f8, tag="a")
                    hw_ = kg * half // 2
                    ringA.dma_start(at[:, 0:hw_], ar[g][:, 0:hw_])
                    ringB.dma_start(at[:, hw_:], ar[g][:, hw_:])
                    tiles[g] = at
                    atv = at.rearrange("p (j m) -> p j m", j=kg)
                    for jp in range(kgp):
                        kpi = gi * kgp + jp
                        kp = g * kg + 2 * jp  # first k-block of the pair
                        for ps, o, w in nch:
                            nc.tensor.matmul(
                                ps[:],
                                hs1[:, kp : kp + 2, :],
                                atv[:, 2 * jp : 2 * jp + 2, o : o + w],
                                start=(kpi == 0),
                                stop=(kpi == kb // 2 - 1),
                                perf_mode=DR,
                            )
                    if gi in inject:
                        inject[gi]()
                return tiles

            def elu_z(ps, o, w, dst, corrap, i):
                # dst[:, o:o+w] = elu(ps/16 + corr), fp32
                z = tmp.tile([128, 512], f32, tag="z", name=f"z{i}")
                nc.vector.tensor_scalar(
                    z[:, :w], ps[:], 1.0 / 16.0, corrap, op0=OP.mult, op1=OP.add
                )
                neg = tmp.tile([128, 512], f32, tag="neg", name=f"neg{i}")
                nc.vector.tensor_scalar_min(neg[:, :w], z[:, :w], 0.0)
                ex = tmp.tile([128, 512], f32, tag="ex", name=f"ex{i}")
                nc.scalar.activation(ex[:, :w], neg[:, :w], AF.Exp)
                pm1 = tmp.tile([128, 512], f32, tag="pm1", name=f"pm1{i}")
                nc.vector.tensor_scalar(
                    pm1[:, :w], z[:, :w], 0.0, -1.0, op0=OP.max, op1=OP.add
                )
                nc.vector.tensor_add(dst[:, o : o + w], ex[:, :w], pm1[:, :w])

            x2t = xepool.tile([128, s], f32, tag="xe")
            cs2 = stat.tile([128, 2], f32, tag="cs2")
            hs2 = hs2p.tile([128, kb, D], f8, tag="hs2")
            x2hs = {}
            cshl = {}

            def half_soft(hx, psn):
                # elu + colsum partial (hi/lo e4m3) + fp16 cast, half hx
                def emit():
                    for i, (ps, o, w) in enumerate(psn):
                        elu_z(ps, hx * half + o, w, x2t, corr1s[:, 0:1], i)
                    nc.vector.tensor_reduce(
                        cs2[:, hx : hx + 1],
                        x2t[:, hx * half : (hx + 1) * half],
                        axis=mybir.AxisListType.X,
                        op=OP.add,
                    )
                    x2h = xhpool.tile([128, half], f16, tag="xh", name=f"xh{hx}")
                    nc.vector.tensor_copy(x2h[:], x2t[:, hx * half : (hx + 1) * half])
                    x2hs[hx] = x2h

                return emit

            def half_bounce(hx, h2b, h2f):
                # own-h2 (node-major) + single-DMA bounce + AllGather trigger
                def emit():
                    x2h = x2hs[hx]
                    h2o = h2opool.tile([128, pw], f8, tag="h2o", name=f"h2o{hx}")
                    for c in range(c8):
                        ph = pss.tile([128, D], f32, tag="pss", name=f"h2o_{hx}_{c}")
                        nc.tensor.matmul(
                            ph[:],
                            x2h[:, c * 128 : (c + 1) * 128],
                            w2s[:],
                            start=True,
                            stop=True,
                        )
                        nc.vector.tensor_copy(h2o[:, c * D : (c + 1) * D], ph[:])
                    # hi/lo e4m3 of colsum/2048 written into the payload tile
                    t = stat.tile([128, 1], f32, tag=f"cst{hx}")
                    nc.vector.tensor_scalar_mul(t[:], cs2[:, hx : hx + 1], 1.0 / 2048.0)
                    nc.vector.tensor_copy(h2o[:, c8 * D : c8 * D + 1], t[:])
                    hi32 = stat.tile([128, 1], f32, tag=f"hi32{hx}")
                    nc.vector.tensor_copy(hi32[:], h2o[:, c8 * D : c8 * D + 1])
                    lo = stat.tile([128, 1], f32, tag=f"lo{hx}")
                    nc.vector.tensor_sub(lo[:], t[:], hi32[:])
                    nc.vector.tensor_scalar_mul(
                        h2o[:, c8 * D + 1 : c8 * D + 2], lo[:], 16.0
                    )
                    ringG.dma_start(h2b[:], h2o[:])
                    ringG.collective_compute(
                        "AllGather",
                        OP.bypass,
                        ins=[h2b.opt()],
                        outs=[h2f.opt()],
                        replica_groups=rg,
                    )

                return emit

            def pulls(hx, h2f, slot0):
                for r in range(ncores):
                    ringG.dma_start(
                        hs2[:, slot0 + r * c8 : slot0 + (r + 1) * c8, :],
                        h2f[r * 128 : (r + 1) * 128, 0 : c8 * 128],
                    )

            # ---- layer 1, pass A (adj columns 0:half) ----
            psA = [
                (big.tile([128, w], f32, tag="big", name=f"pA{i}"), o, w)
                for i, (o, w) in enumerate(chunks(half))
            ]
            tilesA = stream_pass(ar0, ap0, psA, {})

            # ---- layer 1, pass B; half-A epilogue injected into its stream ----
            psB = [
                (big.tile([128, w], f32, tag="big", name=f"pB{i}"), o, w)
                for i, (o, w) in enumerate(chunks(half))
            ]
            tilesB = stream_pass(
                ar1, ap1, psB,
                {0: half_soft(0, psA),
                 min(1, ngrp - 1): half_bounce(0, h2bA, h2fA)},
            )

            # half-B elu must be emitted BEFORE layer 2 reuses psB's banks;
            # its gather goes out before the L2 stream so the PE-idle window
            # between the passes absorbs the h2o matmuls
            half_soft(1, psB)()
            half_bounce(1, h2bB, h2fB)()
            # pulls AFTER both triggers: gpsimd is in-order, so a pull waiting
            # on gather A must not sit ahead of gather B's bounce/trigger
            pulls(0, h2fA, 0)
            pulls(1, h2fB, ncores * c8)

            # ---- corr2 assembly (emitted later into the PE stream) ----
            def corr2_assemble():
                for hx, h2f in ((0, h2fA), (1, h2fB)):
                    hv = h2f.rearrange("(r p) q -> q p r", p=128)
                    for t in (0, 1):
                        ringG.dma_start(
                            parts[:, t, hx * ncores : (hx + 1) * ncores],
                            hv[c8 * 128 + t],
                        )
                partsf = stat.tile([128, 2, 2 * ncores], f32, tag="partsf")
                nc.vector.tensor_copy(partsf[:], parts[:])
                ch = stat.tile([128, 2], f32, tag="ch")
                nc.vector.tensor_reduce(
                    ch[:, 0:1], partsf[:, 0, :], axis=mybir.AxisListType.X, op=OP.add
                )
                nc.vector.tensor_reduce(
                    ch[:, 1:2], partsf[:, 1, :], axis=mybir.AxisListType.X, op=OP.add
                )
                # colsum(x2) = 2048*CH + 128*CL = 128*(16*CH + CL)
                csum = stat.tile([128, 1], f32, tag="csum")
                nc.vector.tensor_scalar(
                    csum[:], ch[:, 0:1], 16.0, ch[:, 1:2], op0=OP.mult, op1=OP.add
                )
                pcv = psy.tile([128, 1], f32, tag="psy", name="pcv")
                nc.tensor.matmul(pcv[:], w2s32[:], csum[:], start=True, stop=True)
                # corr2 = 0.5 * 128 * pcv
                nc.vector.tensor_scalar_mul(corr2s[:], pcv[:], 64.0)

            parts = stat.tile([128, 2, 2 * ncores], f8, tag="parts")
            corr2s = stat.tile([128, 1], f32, tag="corr2s")

            # ---- layer 2: two column-half DoubleRow passes; each pass's
            # tail (elu + out layer + log_softmax) overlaps the next stream --
            kgp = kg // 2

            def l2_pass(ar, apool, tiles, nch):
                for g in range(ngrp):
                    if g < adj_bufs:
                        at = tiles[g]
                    else:
                        hw_ = kg * half // 2
                        at = apool.tile([128, kg * half], f8, tag="a")
                        ringA.dma_start(at[:, 0:hw_], ar[g][:, 0:hw_])
                        ringB.dma_start(at[:, hw_:], ar[g][:, hw_:])
                    av = at.rearrange("p (j m) -> p j m", j=kg)
                    for jp in range(kgp):
                        kpi = g * kgp + jp
                        kp = g * kg + 2 * jp
                        for ps, o, w in nch:
                            nc.tensor.matmul(
                                ps[:],
                                hs2[:, kp : kp + 2, :],
                                av[:, 2 * jp : 2 * jp + 2, o : o + w],
                                start=(kpi == 0),
                                stop=(kpi == kb // 2 - 1),
                                perf_mode=DR,
                            )

            x3t = xepool.tile([128, s], f32, tag="xe")
            outr = out.ap().rearrange("(c p) m -> c p m", p=128)

            def tail(hx, nch):
                # stage-wave emission: minimize in-order engine stalls
                base = hx * half
                zs = []
                for i, (ps, o, w) in enumerate(nch):
                    z = tmp.tile([128, 512], f32, tag="z", name=f"tz{hx}_{i}")
                    nc.vector.tensor_scalar(
                        z[:, :w], ps[:], 1.0 / 16.0, corr2s[:, 0:1],
                        op0=OP.mult, op1=OP.add,
                    )
                    zs.append(z)
                negs = []
                for i, (ps, o, w) in enumerate(nch):
                    neg = tmp.tile([128, 512], f32, tag="neg", name=f"tn{hx}_{i}")
                    nc.vector.tensor_scalar_min(neg[:, :w], zs[i][:, :w], 0.0)
                    negs.append(neg)
                exs = []
                for i, (ps, o, w) in enumerate(nch):
                    ex = tmp.tile([128, 512], f32, tag="ex", name=f"te{hx}_{i}")
                    nc.scalar.activation(ex[:, :w], negs[i][:, :w], AF.Exp)
                    exs.append(ex)
                pms = []
                for i, (ps, o, w) in enumerate(nch):
                    pm1 = tmp.tile([128, 512], f32, tag="pm1", name=f"tp{hx}_{i}")
                    nc.vector.tensor_scalar(
                        pm1[:, :w], zs[i][:, :w], 0.0, -1.0, op0=OP.max, op1=OP.add
                    )
                    pms.append(pm1)
                for i, (ps, o, w) in enumerate(nch):
                    nc.vector.tensor_add(
                        x3t[:, base + o : base + o + w], exs[i][:, :w], pms[i][:, :w]
                    )
                crng = list(range(base // 128, (base + half) // 128))
                pyb = psy.tile([128, c8, C], f32, tag="psy", name=f"pyb{hx}")
                for idx, c in enumerate(crng):
                    nc.tensor.matmul(
                        pyb[:, idx, :], x3t[:, c * 128 : (c + 1) * 128], wouts[:],
                        start=(idx == 0), stop=(idx == c8 - 1),
                    )
                zb = outp.tile([128, c8, C], f32, tag="zb", name=f"zb{hx}")
                for idx, c in enumerate(crng):
                    nc.vector.tensor_add(zb[:, idx, :], pyb[:, idx, :], bbs[:])
                zf = zb.rearrange("p c m -> p (c m)")
                negb = tmp.tile([128, c8 * C], f32, tag="neg", name=f"tnb{hx}")
                nc.vector.tensor_scalar_min(negb[:], zf, 0.0)
                eb = tmp.tile([128, c8 * C], f32, tag="ex", name=f"teb{hx}")
                nc.scalar.activation(eb[:], negb[:], AF.Exp)
                pmb = tmp.tile([128, c8 * C], f32, tag="pm1", name=f"tpb{hx}")
                nc.vector.tensor_scalar(pmb[:], zf, 0.0, -1.0, op0=OP.max, op1=OP.add)
                zzb = outp.tile([128, c8, C], f32, tag="zzb", name=f"zzb{hx}")
                nc.vector.tensor_add(zzb.rearrange("p c m -> p (c m)"), eb[:], pmb[:])
                negm = stat.tile([128, c8], f32, tag="negm", name=f"negm{hx}")
                nc.vector.tensor_reduce(
                    negm[:], zzb[:], axis=mybir.AxisListType.X, op=OP.max, negate=True
                )
                ssum = stat.tile([128, c8], f32, tag="ssum", name=f"ssum{hx}")
                es = tmp.tile([128, c8 * C], f32, tag="z", name=f"tes{hx}")
                esv = es.rearrange("p (c m) -> p c m", m=C)
                for idx in range(c8):
                    nc.scalar.activation(
                        esv[:, idx, :],
                        zzb[:, idx, :],
                        AF.Exp,
                        bias=negm[:, idx : idx + 1],
                        accum_out=ssum[:, idx : idx + 1],
                    )
                lse = stat.tile([128, c8], f32, tag="lse", name=f"lse{hx}")
                nc.scalar.activation(lse[:], ssum[:], AF.Ln)
                osbs = outp.tile([128, c8, C], f32, tag="osb", name=f"osb{hx}")
                for idx, c in enumerate(crng):
                    nc.vector.tensor_scalar(
                        osbs[:, idx, :],
                        zzb[:, idx, :],
                        negm[:, idx : idx + 1],
                        lse[:, idx : idx + 1],
                        op0=OP.add,
                        op1=OP.subtract,
                    )
                for idx, c in enumerate(crng):
                    ringG.dma_start(outr[c], osbs[:, idx, :])

            # pass alpha: own nodes 0:half (adjT0), full k
            psLa = [
                (big.tile([128, w], f32, tag="big", name=f"pLa{i}"), o, w)
                for i, (o, w) in enumerate(chunks(half))
            ]
            l2_pass(ar0, ap0, tilesA, psLa)
            corr2_assemble()
            # pass beta: own nodes half:s (adjT1); alpha tail overlaps it
            psLb = [
                (big.tile([128, w], f32, tag="big", name=f"pLb{i}"), o, w)
                for i, (o, w) in enumerate(chunks(half))
            ]
            tail(0, psLa)
            l2_pass(ar1, ap1, tilesB, psLb)
            tail(1, psLb)

    nc.compile()
    return nc


def make_in_maps(x, adj, W1, W2, Wout, bout, ncores=NCORES, kg=8):
    import ml_dtypes

    f8np = ml_dtypes.float8_e4m3
    n_total = adj.shape[0]
    s = n_total // ncores
    half = s // 2
    kb = n_total // 128
    ngrp = kb // kg
    kb8 = kb // 8
    f, d = W1.shape[1], W1.shape[0] * W1.shape[2]

    # global contraction-row order: all ranks' first halves, then seconds
    perm = np.concatenate(
        [np.arange(r * s, r * s + half) for r in range(ncores)]
        + [np.arange(r * s + half, (r + 1) * s) for r in range(ncores)]
    )

    w1f32 = np.ascontiguousarray(W1.transpose(1, 0, 2).reshape(f, d).astype(np.float32))
    w1f = w1f32.astype(f8np)
    w2f = np.ascontiguousarray(W2.transpose(1, 0, 2).reshape(d, d).astype(np.float16))
    woutf = np.ascontiguousarray(Wout.astype(np.float32))
    bbf = np.ascontiguousarray(
        np.broadcast_to(bout.astype(np.float32), (128, Wout.shape[1]))
    )
    # corr1 = 0.5 * colsum(h1) = 0.5 * colsum(x) @ W1f
    corr1 = (0.5 * (x.astype(np.float32).sum(0) @ w1f32)).astype(np.float32)
    corr1 = np.ascontiguousarray(corr1.reshape(d, 1))

    # fp8 e4m3 residual of adj, scaled by 16
    r8 = ((adj.astype(np.float32) - 0.5) * 16.0).astype(f8np)

    # xc[g*128 + p, ((j*2 + a)*128) + m] = xperm.T[a*128 + p, (g*8 + j)*128 + m]
    x8 = x.astype(f8np)
    xtc = x8[perm].T  # [F, n_total]
    xcf = np.ascontiguousarray(
        xtc.reshape(2, 128, kb8, 8, 128)
        .transpose(2, 1, 3, 0, 4)
        .reshape(kb8 * 128, 8 * f)
    )

    def reorder(t):
        # dram row (g*kg*128 + p*kg + j) <- k-row (g*kg*128 + j*128 + p)
        return np.ascontiguousarray(
            t.reshape(ngrp, kg, 128, t.shape[1]).transpose(0, 2, 1, 3).reshape(t.shape)
        )

    in_maps = []
    for c in range(ncores):
        t0 = r8[c * s : c * s + half, :][:, perm].T  # [n_total, half]
        t1 = r8[c * s + half : (c + 1) * s, :][:, perm].T
        in_maps.append(
            {
                "adjt0": reorder(t0),
                "adjt1": reorder(t1),
                "xc": xcf,
                "w1": w1f,
                "w2": w2f,
                "wout": woutf,
                "bb": bbf,
                "corr1": corr1,
            }
        )
    return in_maps


def kernel(x, adj, W1, W2, Wout, bout):
    from concourse import bass_utils

    x = np.asarray(x)
    adj = np.asarray(adj)
    in_maps = make_in_maps(x, adj, np.asarray(W1), np.asarray(W2),
                           np.asarray(Wout), np.asarray(bout))
    if "nc" not in _nc_cache:
        _nc_cache["nc"] = build_gat_nc()
    res = bass_utils.run_bass_kernel_spmd(
        _nc_cache["nc"], in_maps, core_ids=list(range(NCORES))
    )
    return np.concatenate([r["out"] for r in res.results], axis=0).astype(np.float32)


# revision 29
# speedup vs baseline: 1.2186x; 1.2186x over previous
"""Bass/Trainium2 kernel for the (dead-attention) GAT reference.

Effective math (see reference):
    h1  = x @ W1f                 W1f = W1.transpose(1,0,2).reshape(256,128)
    hp1 = elu(adj @ h1)
    h2  = hp1 @ W2f               W2f = W2.transpose(1,0,2).reshape(128,128)
    hp2 = elu(adj @ h2)
    y   = elu(hp2 @ Wout + bout)
    out = log_softmax(y, axis=1)

v3 design:
  * adj stored as fp8 e4m3 residual  R16 = 16*(adj - 0.5)  and h1/h2
    stationaries in e4m3, so every big matmul runs in DoubleRow perf
    mode: one instruction contracts a PAIR of 128-row k-blocks at 0.5
    cycles/row.  This halves HBM traffic (vs fp16) and roughly halves
    PE instruction count/stream time.  The rank-1 term 0.5*J@h lost by
    the residual encoding is restored per feature when reading PSUM:
        hp = elu(psum/16 + 0.5*colsum(h))
    corr1 = 0.5*colsum(x)@W1f is a host-prepared constant; corr2 is
    assembled from per-core colsum(x2) partials carried in the gather
    payloads as e4m3 hi/lo pairs, then one on-device fp32 matvec.
    Verified numerics vs fp32 reference (host emulation): 3.1e-4.
  * adj rows (contraction dim) use a GLOBAL order (all ranks' first-
    half nodes, then all ranks' second halves), identical on every
    core.  Layer 1 runs as two column-half passes; each pass's h2 half
    is AllGathered while subsequent compute streams, hiding the
    collectives.  Payloads are partition-major (one DMA line per
    partition) and gathered blocks DMA straight into the node-major
    stationary buffer hs2 with one plain 2D copy per rank - no PE
    transposes, no dynamic offsets.
  * Layer-1 passes stream their adj groups in REVERSED order so the
    last `adj_bufs` group tiles of both column halves stay resident in
    SBUF and layer 2 re-reads that much less adj from HBM.
"""

import sys

import numpy as np

sys.path.insert(0, "/opt/trn_rl_repo")

N = 16384  # nodes
F = 256  # input features
D = 128  # hidden width (nheads*nhid)
C = 32  # classes
NCORES = 8
S = N // NCORES  # rows per core

_nc_cache = {}


def build_gat_nc(n_total=N, ncores=NCORES, enable_asserts=False, adj_bufs=7, kg=8):
    """Build the SPMD Bass program (one program, runs on all cores)."""
    from concourse import bacc, mybir, tile

    s = n_total // ncores  # own rows per core
    half = s // 2  # column-half width
    kb = n_total // 128  # contraction blocks
    ngrp = kb // kg  # adj DMA groups
    kb8 = kb // 8  # xc chunk groups
    c8 = half // 128  # 128-node blocks per (rank, half)
    rc = s // 128  # 128-row chunks of the own shard
    f32 = mybir.dt.float32
    f16 = mybir.dt.float16
    f8 = mybir.dt.float8e4
    AF = mybir.ActivationFunctionType
    OP = mybir.AluOpType
    DR = mybir.MatmulPerfMode.DoubleRow

    def chunks(width):
        return [(o, min(512, width - o)) for o in range(0, width, 512)]

    nc = bacc.Bacc(
        "TRN2",
        target_bir_lowering=False,
        debug=False,
        enable_asserts=enable_asserts,
        num_devices=ncores,
    )

    adjt0 = nc.dram_tensor("adjt0", [n_total, half], f8, kind="ExternalInput")
    adjt1 = nc.dram_tensor("adjt1", [n_total, half], f8, kind="ExternalInput")
    xc = nc.dram_tensor("xc", [kb8 * 128, 8 * F], f8, kind="ExternalInput")
    w1 = nc.dram_tensor("w1", [F, D], f8, kind="ExternalInput")
    w2 = nc.dram_tensor("w2", [D, D], f16, kind="ExternalInput")
    wout = nc.dram_tensor("wout", [D, C], f32, kind="ExternalInput")
    bb = nc.dram_tensor("bb", [128, C], f32, kind="ExternalInput")
    corr1 = nc.dram_tensor("corr1", [128, 1], f32, kind="ExternalInput")
    out = nc.dram_tensor("out", [s, C], f32, kind="ExternalOutput")

    rg = [list(range(ncores))]
    pw = c8 * 128 + 2  # payload width: c8 h2 blocks + colsum hi/lo

    with tile.TileContext(nc) as tc:
        with (
            tc.tile_pool(name="dram", bufs=1, space="DRAM") as dram,
            tc.tile_pool(name="const", bufs=1) as const,
            tc.tile_pool(name="hs1p", bufs=1) as hs1p,
            tc.tile_pool(name="hs2p", bufs=1) as hs2p,
            tc.tile_pool(name="ap0", bufs=adj_bufs) as ap0,
            tc.tile_pool(name="ap1", bufs=adj_bufs) as ap1,
            tc.tile_pool(name="xcp", bufs=4) as xcpool,
            tc.tile_pool(name="xe", bufs=2) as xepool,
            tc.tile_pool(name="xh", bufs=2) as xhpool,
            tc.tile_pool(name="h2o", bufs=2) as h2opool,
            tc.tile_pool(name="tmp", bufs=1) as tmp,
            tc.tile_pool(name="outp", bufs=2) as outp,
            tc.tile_pool(name="stat", bufs=1) as stat,
            tc.tile_pool(name="big", bufs=4, space="PSUM") as big,
            tc.tile_pool(name="pss", bufs=2, space="PSUM") as pss,
            tc.tile_pool(name="psy", bufs=2, space="PSUM") as psy,
        ):
            # ring assignment:
            #   A/B (sync + scalar HWDGE): xc and the big adj streams only
            #   G (gpsimd SWDGE): constants, bounces, collective triggers,
            #     gather pulls, output stores.  NOTE: keep collective-output
            #     reads off the sync ring (test_sync_dma_collective_hang).
            ringA, ringB = nc.sync, nc.scalar
            ringG = nc.gpsimd

            # --- replicated constants ---
            w1s = const.tile([128, 2, D], f8, tag="w1s")
            ringG.dma_start(w1s[:], w1.ap().rearrange("(a p) m -> p a m", p=128))
            w2s = const.tile([128, D], f16, tag="w2s")
            ringG.dma_start(w2s[:], w2.ap())
            wouts = const.tile([128, C], f32, tag="wouts")
            ringG.dma_start(wouts[:], wout.ap())
            bbs = const.tile([128, C], f32, tag="bbs")
            ringG.dma_start(bbs[:], bb.ap())
            corr1s = const.tile([128, 1], f32, tag="corr1s")
            ringG.dma_start(corr1s[:], corr1.ap())
            # fp32 copy of W2f for the (fp32) corr2 matvec
            w2s32 = const.tile([128, D], f32, tag="w2s32")
            nc.vector.tensor_copy(w2s32[:], w2s[:])

            # --- tiny warm-up collective: absorbs the CC-stream wakeup
            # latency (~11us) while the stream is otherwise idle ---
            zs = const.tile([1, 64], f8, tag="zs")
            nc.vector.memset(zs[:], 0.0)
            dumb = dram.tile([1, 64], f8, tag="dumb")
            dumf = dram.tile([ncores, 64], f8, tag="dumf", addr_space="Shared")
            ringG.dma_start(dumb[:], zs[:])
            ringG.collective_compute(
                "AllGather", OP.bypass, ins=[dumb.opt()], outs=[dumf.opt()],
                replica_groups=rg,
            )

            # --- DRAM bounce buffers for the collectives (partition-major) ---
            h2bA = dram.tile([128, pw], f8, tag="h2bA")
            h2bB = dram.tile([128, pw], f8, tag="h2bB")
            h2fA = dram.tile([ncores * 128, pw], f8, tag="h2fA", addr_space="Shared")
            h2fB = dram.tile([ncores * 128, pw], f8, tag="h2fB", addr_space="Shared")

            # ---- h1 = x @ W1f, replicated (DoubleRow over the two F-halves) ----
            # xc group g holds 8 chunk-lhsTs contiguous per partition:
            # xc[g*128+p, ((j*2+a)*128)+m] = xperm.T[a*128+p, (g*8+j)*128+m]
            hs1 = hs1p.tile([128, kb, D], f8, tag="hs1")
            xr = xc.ap().rearrange("(g p) q -> g p q", p=128)
            xg = None
            for k4 in range(kb // 4):
                ph = big.tile([128, 4, D], f32, tag="big", name=f"ph1_{k4}")
                for i in range(4):
                    k = k4 * 4 + i
                    g, j = divmod(k, 8)
                    if j == 0:
                        xg = xcpool.tile([128, 8, 2, 128], f8, tag="xg")
                        (ringA if g % 2 == 0 else ringB).dma_start(
                            xg.rearrange("p j a m -> p (j a m)"), xr[g]
                        )
                    nc.tensor.matmul(
                        ph[:, i, :], xg[:, j, :, :], w1s[:],
                        start=(i == 0), stop=(i == 3), perf_mode=DR,
                    )
                nc.vector.tensor_copy(hs1[:, k4 * 4 : (k4 + 1) * 4, :], ph[:])

            ar0 = adjt0.ap().rearrange("(g p j) m -> g p (j m)", p=128, j=kg)
            ar1 = adjt1.ap().rearrange("(g p j) m -> g p (j m)", p=128, j=kg)

            def stream_pass(ar, apool, nch, inject):
                """One reversed-order DoubleRow streaming pass over `ar`.

                nch: list of (psum_tile, o, w) output chunks.
                inject: {group_iteration_index: callback} runs after that
                group's matmuls are emitted.
                Returns {g: sbuf tile}; entries g < adj_bufs stay resident.
                """
                tiles = {}
                kgp = kg // 2
                for gi, g in enumerate(reversed(range(ngrp))):
                    at = apool.tile([128, kg * half], f8, tag="a")
                    hw_ = kg * half // 2
                    ringA.dma_start(at[:, 0:hw_], ar[g][:, 0:hw_])
                    ringB.dma_start(at[:, hw_:], ar[g][:, hw_:])
                    tiles[g] = at
                    atv = at.rearrange("p (j m) -> p j m", j=kg)
                    for jp in range(kgp):
                        kpi = gi * kgp + jp
                        kp = g * kg + 2 * jp  # first k-block of the pair
                        for ps, o, w in nch:
                            nc.tensor.matmul(
                                ps[:],
                                hs1[:, kp : kp + 2, :],
                                atv[:, 2 * jp : 2 * jp + 2, o : o + w],
                                start=(kpi == 0),
                                stop=(kpi == kb // 2 - 1),
                                perf_mode=DR,
                            )
                    if gi in inject:
                        inject[gi]()
                return tiles

            def elu_z(ps, o, w, dst, corrap, i):
                # dst[:, o:o+w] = elu(ps/16 + corr), fp32
                z = tmp.tile([128, 512], f32, tag="z", name=f"z{i}")
                nc.vector.tensor_scalar(
                    z[:, :w], ps[:], 1.0 / 16.0, corrap, op0=OP.mult, op1=OP.add
                )
                neg = tmp.tile([128, 512], f32, tag="neg", name=f"neg{i}")
                nc.vector.tensor_scalar_min(neg[:, :w], z[:, :w], 0.0)
                ex = tmp.tile([128, 512], f32, tag="ex", name=f"ex{i}")
                nc.scalar.activation(ex[:, :w], neg[:, :w], AF.Exp)
                pm1 = tmp.tile([128, 512], f32, tag="pm1", name=f"pm1{i}")
                nc.vector.tensor_scalar(
                    pm1[:, :w], z[:, :w], 0.0, -1.0, op0=OP.max, op1=OP.add
                )
                nc.vector.tensor_add(dst[:, o : o + w], ex[:, :w], pm1[:, :w])

            x2t = xepool.tile([128, s], f32, tag="xe")
            cs2 = stat.tile([128, 2], f32, tag="cs2")
            hs2 = hs2p.tile([128, kb, D], f8, tag="hs2")
            x2hs = {}
            cshl = {}

            def half_soft(hx, psn):
                # elu + colsum partial (hi/lo e4m3) + fp16 cast, half hx
                def emit():
                    for i, (ps, o, w) in enumerate(psn):
                        elu_z(ps, hx * half + o, w, x2t, corr1s[:, 0:1], i)
                    nc.vector.tensor_reduce(
                        cs2[:, hx : hx + 1],
                        x2t[:, hx * half : (hx + 1) * half],
                        axis=mybir.AxisListType.X,
                        op=OP.add,
                    )
                    x2h = xhpool.tile([128, half], f16, tag="xh", name=f"xh{hx}")
                    nc.vector.tensor_copy(x2h[:], x2t[:, hx * half : (hx + 1) * half])
                    x2hs[hx] = x2h

                return emit

            def half_bounce(hx, h2b, h2f):
                # own-h2 (node-major) + single-DMA bounce + AllGather trigger
                def emit():
                    x2h = x2hs[hx]
                    h2o = h2opool.tile([128, pw], f8, tag="h2o", name=f"h2o{hx}")
                    for c in range(c8):
                        ph = pss.tile([128, D], f32, tag="pss", name=f"h2o_{hx}_{c}")
                        nc.tensor.matmul(
                            ph[:],
                            x2h[:, c * 128 : (c + 1) * 128],
                            w2s[:],
                            start=True,
                            stop=True,
                        )
                        nc.vector.tensor_copy(h2o[:, c * D : (c + 1) * D], ph[:])
                    # hi/lo e4m3 of colsum/2048 written into the payload tile
                    t = stat.tile([128, 1], f32, tag=f"cst{hx}")
                    nc.vector.tensor_scalar_mul(t[:], cs2[:, hx : hx + 1], 1.0 / 2048.0)
                    nc.vector.tensor_copy(h2o[:, c8 * D : c8 * D + 1], t[:])
                    hi32 = stat.tile([128, 1], f32, tag=f"hi32{hx}")
                    nc.vector.tensor_copy(hi32[:], h2o[:, c8 * D : c8 * D + 1])
                    lo = stat.tile([128, 1], f32, tag=f"lo{hx}")
                    nc.vector.tensor_sub(lo[:], t[:], hi32[:])
                    nc.vector.tensor_scalar_mul(
                        h2o[:, c8 * D + 1 : c8 * D + 2], lo[:], 16.0
                    )
                    ringG.dma_start(h2b[:], h2o[:])
                    ringG.collective_compute(
                        "AllGather",
                        OP.bypass,
                        ins=[h2b.opt()],
                        outs=[h2f.opt()],
                        replica_groups=rg,
                    )

                return emit

            def pulls(hx, h2f, slot0):
                for r in range(ncores):
                    ringG.dma_start(
                        hs2[:, slot0 + r * c8 : slot0 + (r + 1) * c8, :],
                        h2f[r * 128 : (r + 1) * 128, 0 : c8 * 128],
                    )

            # ---- layer 1, pass A (adj columns 0:half) ----
            psA = [
                (big.tile([128, w], f32, tag="big", name=f"pA{i}"), o, w)
                for i, (o, w) in enumerate(chunks(half))
            ]
            tilesA = stream_pass(ar0, ap0, psA, {})

            # ---- layer 1, pass B; half-A epilogue injected into its stream ----
            psB = [
                (big.tile([128, w], f32, tag="big", name=f"pB{i}"), o, w)
                for i, (o, w) in enumerate(chunks(half))
            ]
            tilesB = stream_pass(
                ar1, ap1, psB,
                {0: half_soft(0, psA),
                 min(1, ngrp - 1): half_bounce(0, h2bA, h2fA)},
            )

            # half-B elu must be emitted BEFORE layer 2 reuses psB's banks;
            # its gather goes out before the L2 stream so the PE-idle window
            # between the passes absorbs the h2o matmuls
            half_soft(1, psB)()
            half_bounce(1, h2bB, h2fB)()
            # pulls AFTER both triggers: gpsimd is in-order, so a pull waiting
            # on gather A must not sit ahead of gather B's bounce/trigger
            pulls(0, h2fA, 0)
            pulls(1, h2fB, ncores * c8)

            # ---- corr2 assembly (emitted later into the PE stream) ----
            def corr2_assemble():
                for hx, h2f in ((0, h2fA), (1, h2fB)):
                    hv = h2f.rearrange("(r p) q -> q p r", p=128)
                    for t in (0, 1):
                        ringG.dma_start(
                            parts[:, t, hx * ncores : (hx + 1) * ncores],
                            hv[c8 * 128 + t],
                        )
                partsf = stat.tile([128, 2, 2 * ncores], f32, tag="partsf")
                nc.vector.tensor_copy(partsf[:], parts[:])
                ch = stat.tile([128, 2], f32, tag="ch")
                nc.vector.tensor_reduce(
                    ch[:, 0:1], partsf[:, 0, :], axis=mybir.AxisListType.X, op=OP.add
                )
                nc.vector.tensor_reduce(
                    ch[:, 1:2], partsf[:, 1, :], axis=mybir.AxisListType.X, op=OP.add
                )
                # colsum(x2) = 2048*CH + 128*CL = 128*(16*CH + CL)
                csum = stat.tile([128, 1], f32, tag="csum")
                nc.vector.tensor_scalar(
                    csum[:], ch[:, 0:1], 16.0, ch[:, 1:2], op0=OP.mult, op1=OP.add
                )
                pcv = psy.tile([128, 1], f32, tag="psy", name="pcv")
                nc.tensor.matmul(pcv[:], w2s32[:], csum[:], start=True, stop=True)
                # corr2 = 0.5 * 128 * pcv
                nc.vector.tensor_scalar_mul(corr2s[:], pcv[:], 64.0)

            parts = stat.tile([128, 2, 2 * ncores], f8, tag="parts")
            corr2s = stat.tile([128, 1], f32, tag="corr2s")

            # ---- layer 2: two column-half DoubleRow passes; each pass's
            # tail (elu + out layer + log_softmax) overlaps the next stream --
            kgp = kg // 2

            def l2_pass(ar, apool, tiles, nch):
                for g in range(ngrp):
                    if g < adj_bufs:
                        at = tiles[g]
                    else:
                        hw_ = kg * half // 2
                        at = apool.tile([128, kg * half], f8, tag="a")
                        ringA.dma_start(at[:, 0:hw_], ar[g][:, 0:hw_])
                        ringB.dma_start(at[:, hw_:], ar[g][:, hw_:])
                    av = at.rearrange("p (j m) -> p j m", j=kg)
                    for jp in range(kgp):
                        kpi = g * kgp + jp
                        kp = g * kg + 2 * jp
                        for ps, o, w in nch:
                            nc.tensor.matmul(
                                ps[:],
                                hs2[:, kp : kp + 2, :],
                                av[:, 2 * jp : 2 * jp + 2, o : o + w],
                                start=(kpi == 0),
                                stop=(kpi == kb // 2 - 1),
                                perf_mode=DR,
                            )

            x3t = xepool.tile([128, s], f32, tag="xe")
            outr = out.ap().rearrange("(c p) m -> c p m", p=128)

            def tail(hx, nch):
                # stage-wave emission: minimize in-order engine stalls
                base = hx * half
                zs = []
                for i, (ps, o, w) in enumerate(nch):
                    z = tmp.tile([128, 512], f32, tag="z", name=f"tz{hx}_{i}")
                    nc.vector.tensor_scalar(
                        z[:, :w], ps[:], 1.0 / 16.0, corr2s[:, 0:1],
                        op0=OP.mult, op1=OP.add,
                    )
                    zs.append(z)
                negs = []
                for i, (ps, o, w) in enumerate(nch):
                    neg = tmp.tile([128, 512], f32, tag="neg", name=f"tn{hx}_{i}")
                    nc.vector.tensor_scalar_min(neg[:, :w], zs[i][:, :w], 0.0)
                    negs.append(neg)
                exs = []
                for i, (ps, o, w) in enumerate(nch):
                    ex = tmp.tile([128, 512], f32, tag="ex", name=f"te{hx}_{i}")
                    nc.scalar.activation(ex[:, :w], negs[i][:, :w], AF.Exp)
                    exs.append(ex)
                pms = []
                for i, (ps, o, w) in enumerate(nch):
                    pm1 = tmp.tile([128, 512], f32, tag="pm1", name=f"tp{hx}_{i}")
                    nc.vector.tensor_scalar(
                        pm1[:, :w], zs[i][:, :w], 0.0, -1.0, op0=OP.max, op1=OP.add
                    )
                    pms.append(pm1)
                for i, (ps, o, w) in enumerate(nch):
                    nc.vector.tensor_add(
                        x3t[:, base + o : base + o + w], exs[i][:, :w], pms[i][:, :w]
                    )
                crng = list(range(base // 128, (base + half) // 128))
                pyb = psy.tile([128, c8, C], f32, tag="psy", name=f"pyb{hx}")
                for idx, c in enumerate(crng):
                    nc.tensor.matmul(
                        pyb[:, idx, :], x3t[:, c * 128 : (c + 1) * 128], wouts[:],
                        start=(idx == 0), stop=(idx == c8 - 1),
                    )
                zb = outp.tile([128, c8, C], f32, tag="zb", name=f"zb{hx}")
                for idx, c in enumerate(crng):
                    nc.vector.tensor_add(zb[:, idx, :], pyb[:, idx, :], bbs[:])
                zf = zb.rearrange("p c m -> p (c m)")
                negb = tmp.tile([128, c8 * C], f32, tag="neg", name=f"tnb{hx}")
                nc.vector.tensor_scalar_min(negb[:], zf, 0.0)
                eb = tmp.tile([128, c8 * C], f32, tag="ex", name=f"teb{hx}")
                nc.scalar.activation(eb[:], negb[:], AF.Exp)
                pmb = tmp.tile([128, c8 * C], f32, tag="pm1", name=f"tpb{hx}")
                nc.vector.tensor_scalar(pmb[:], zf, 0.0, -1.0, op0=OP.max, op1=OP.add)
                zzb = outp.tile([128, c8, C], f32, tag="zzb", name=f"zzb{hx}")
                nc.vector.tensor_add(zzb.rearrange("p c m -> p (c m)"), eb[:], pmb[:])
                negm = stat.tile([128, c8], f32, tag="negm", name=f"negm{hx}")
                nc.vector.tensor_reduce(
                    negm[:], zzb[:], axis=mybir.AxisListType.X, op=OP.max, negate=True
                )
                ssum = stat.tile([128, c8], f32, tag="ssum", name=f"ssum{hx}")
                es = tmp.tile([128, c8 * C], f32, tag="z", name=f"tes{hx}")
                esv = es.rearrange("p (c m) -> p c m", m=C)
                for idx in range(c8):
                    nc.scalar.activation(
                        esv[:, idx, :],
                        zzb[:, idx, :],
                        AF.Exp,
                        bias=negm[:, idx : idx + 1],
                        accum_out=ssum[:, idx : idx + 1],
                    )
                lse = stat.tile([128, c8], f32, tag="lse", name=f"lse{hx}")
                nc.scalar.activation(lse[:], ssum[:], AF.Ln)
                osbs = outp.tile([128, c8, C], f32, tag="osb", name=f"osb{hx}")
                for idx, c in enumerate(crng):
                    nc.vector.tensor_scalar(
                        osbs[:, idx, :],
                        zzb[:, idx, :],
                        negm[:, idx : idx + 1],
                        lse[:, idx : idx + 1],
                        op0=OP.add,
                        op1=OP.subtract,
                    )
                for idx, c in enumerate(crng):
                    ringG.dma_start(outr[c], osbs[:, idx, :])

            # pass alpha: own nodes 0:half (adjT0), full k
            psLa = [
                (big.tile([128, w], f32, tag="big", name=f"pLa{i}"), o, w)
                for i, (o, w) in enumerate(chunks(half))
            ]
            l2_pass(ar0, ap0, tilesA, psLa)
            corr2_assemble()
            # pass beta: own nodes half:s (adjT1); alpha tail overlaps it
            psLb = [
                (big.tile([128, w], f32, tag="big", name=f"pLb{i}"), o, w)
                for i, (o, w) in enumerate(chunks(half))
            ]
            tail(0, psLa)
            l2_pass(ar1, ap1, tilesB, psLb)
            tail(1, psLb)

    nc.compile()
    return nc


def make_in_maps(x, adj, W1, W2, Wout, bout, ncores=NCORES, kg=8):
    import ml_dtypes

    f8np = ml_dtypes.float8_e4m3
    n_total = adj.shape[0]
    s = n_total // ncores
    half = s // 2
    kb = n_total // 128
    ngrp = kb // kg
    kb8 = kb // 8
    f, d = W1.shape[1], W1.shape[0] * W1.shape[2]

    # global contraction-row order: all ranks' first halves, then seconds
    perm = np.concatenate(
        [np.arange(r * s, r * s + half) for r in range(ncores)]
        + [np.arange(r * s + half, (r + 1) * s) for r in range(ncores)]
    )

    w1f32 = np.ascontiguousarray(W1.transpose(1, 0, 2).reshape(f, d).astype(np.float32))
    w1f = w1f32.astype(f8np)
    w2f = np.ascontiguousarray(W2.transpose(1, 0, 2).reshape(d, d).astype(np.float16))
    woutf = np.ascontiguousarray(Wout.astype(np.float32))
    bbf = np.ascontiguousarray(
        np.broadcast_to(bout.astype(np.float32), (128, Wout.shape[1]))
    )
    # corr1 = 0.5 * colsum(h1) = 0.5 * colsum(x) @ W1f
    corr1 = (0.5 * (x.astype(np.float32).sum(0) @ w1f32)).astype(np.float32)
    corr1 = np.ascontiguousarray(corr1.reshape(d, 1))

    # fp8 e4m3 residual of adj, scaled by 16
    r8 = ((adj.astype(np.float32) - 0.5) * 16.0).astype(f8np)

    # xc[g*128 + p, ((j*2 + a)*128) + m] = xperm.T[a*128 + p, (g*8 + j)*128 + m]
    x8 = x.astype(f8np)
    xtc = x8[perm].T  # [F, n_total]
    xcf = np.ascontiguousarray(
        xtc.reshape(2, 128, kb8, 8, 128)
        .transpose(2, 1, 3, 0, 4)
        .reshape(kb8 * 128, 8 * f)
    )

    def reorder(t):
        # dram row (g*kg*128 + p*kg + j) <- k-row (g*kg*128 + j*128 + p)
        return np.ascontiguousarray(
            t.reshape(ngrp, kg, 128, t.shape[1]).transpose(0, 2, 1, 3).reshape(t.shape)
        )

    in_maps = []
    for c in range(ncores):
        t0 = r8[c * s : c * s + half, :][:, perm].T  # [n_total, half]
        t1 = r8[c * s + half : (c + 1) * s, :][:, perm].T
        in_maps.append(
            {
                "adjt0": reorder(t0),
                "adjt1": reorder(t1),
                "xc": xcf,
                "w1": w1f,
                "w2": w2f,
                "wout": woutf,
                "bb": bbf,
                "corr1": corr1,
            }
        )
    return in_maps


def kernel(x, adj, W1, W2, Wout, bout):
    from concourse import bass_utils

    x = np.asarray(x)
    adj = np.asarray(adj)
    in_maps = make_in_maps(x, adj, np.asarray(W1), np.asarray(W2),
                           np.asarray(Wout), np.asarray(bout))
    if "nc" not in _nc_cache:
        _nc_cache["nc"] = build_gat_nc()
    res = bass_utils.run_bass_kernel_spmd(
        _nc_cache["nc"], in_maps, core_ids=list(range(NCORES))
    )
    return np.concatenate([r["out"] for r in res.results], axis=0).astype(np.float32)


# revision 30
# speedup vs baseline: 1.2203x; 1.0014x over previous
"""Bass/Trainium2 kernel for the (dead-attention) GAT reference.

Effective math (see reference):
    h1  = x @ W1f                 W1f = W1.transpose(1,0,2).reshape(256,128)
    hp1 = elu(adj @ h1)
    h2  = hp1 @ W2f               W2f = W2.transpose(1,0,2).reshape(128,128)
    hp2 = elu(adj @ h2)
    y   = elu(hp2 @ Wout + bout)
    out = log_softmax(y, axis=1)

v3 design:
  * adj stored as fp8 e4m3 residual  R16 = 16*(adj - 0.5)  and h1/h2
    stationaries in e4m3, so every big matmul runs in DoubleRow perf
    mode: one instruction contracts a PAIR of 128-row k-blocks at 0.5
    cycles/row.  This halves HBM traffic (vs fp16) and roughly halves
    PE instruction count/stream time.  The rank-1 term 0.5*J@h lost by
    the residual encoding is restored per feature when reading PSUM:
        hp = elu(psum/16 + 0.5*colsum(h))
    corr1 = 0.5*colsum(x)@W1f is a host-prepared constant; corr2 is
    assembled from per-core colsum(x2) partials carried in the gather
    payloads as e4m3 hi/lo pairs, then one on-device fp32 matvec.
    Verified numerics vs fp32 reference (host emulation): 3.1e-4.
  * adj rows (contraction dim) use a GLOBAL order (all ranks' first-
    half nodes, then all ranks' second halves), identical on every
    core.  Layer 1 runs as two column-half passes; each pass's h2 half
    is AllGathered while subsequent compute streams, hiding the
    collectives.  Payloads are partition-major (one DMA line per
    partition) and gathered blocks DMA straight into the node-major
    stationary buffer hs2 with one plain 2D copy per rank - no PE
    transposes, no dynamic offsets.
  * Layer-1 passes stream their adj groups in REVERSED order so the
    last `adj_bufs` group tiles of both column halves stay resident in
    SBUF and layer 2 re-reads that much less adj from HBM.
"""

import sys

import numpy as np

sys.path.insert(0, "/opt/trn_rl_repo")

N = 16384  # nodes
F = 256  # input features
D = 128  # hidden width (nheads*nhid)
C = 32  # classes
NCORES = 8
S = N // NCORES  # rows per core

_nc_cache = {}


def build_gat_nc(n_total=N, ncores=NCORES, enable_asserts=False, adj_bufs=6, kg=8):
    """Build the SPMD Bass program (one program, runs on all cores)."""
    from concourse import bacc, mybir, tile

    s = n_total // ncores  # own rows per core
    half = s // 2  # column-half width
    kb = n_total // 128  # contraction blocks
    ngrp = kb // kg  # adj DMA groups
    kb8 = kb // 8  # xc chunk groups
    c8 = half // 128  # 128-node blocks per (rank, half)
    rc = s // 128  # 128-row chunks of the own shard
    f32 = mybir.dt.float32
    f16 = mybir.dt.float16
    f8 = mybir.dt.float8e4
    AF = mybir.ActivationFunctionType
    OP = mybir.AluOpType
    DR = mybir.MatmulPerfMode.DoubleRow

    def chunks(width):
        return [(o, min(512, width - o)) for o in range(0, width, 512)]

    nc = bacc.Bacc(
        "TRN2",
        target_bir_lowering=False,
        debug=False,
        enable_asserts=enable_asserts,
        num_devices=ncores,
    )

    adjt0 = nc.dram_tensor("adjt0", [n_total, half], f8, kind="ExternalInput")
    adjt1 = nc.dram_tensor("adjt1", [n_total, half], f8, kind="ExternalInput")
    xc = nc.dram_tensor("xc", [kb8 * 128, 8 * F], f8, kind="ExternalInput")
    w1 = nc.dram_tensor("w1", [F, D], f8, kind="ExternalInput")
    w2 = nc.dram_tensor("w2", [D, D], f16, kind="ExternalInput")
    wout = nc.dram_tensor("wout", [D, C], f32, kind="ExternalInput")
    bb = nc.dram_tensor("bb", [128, C], f32, kind="ExternalInput")
    corr1 = nc.dram_tensor("corr1", [128, 1], f32, kind="ExternalInput")
    out = nc.dram_tensor("out", [s, C], f32, kind="ExternalOutput")

    rg = [list(range(ncores))]
    pw = c8 * 128 + 2  # payload width: c8 h2 blocks + colsum hi/lo

    with tile.TileContext(nc) as tc:
        with (
            tc.tile_pool(name="dram", bufs=1, space="DRAM") as dram,
            tc.tile_pool(name="const", bufs=1) as const,
            tc.tile_pool(name="hs1p", bufs=1) as hs1p,
            tc.tile_pool(name="hs2p", bufs=1) as hs2p,
            tc.tile_pool(name="ap0", bufs=adj_bufs) as ap0,
            tc.tile_pool(name="ap1", bufs=adj_bufs) as ap1,
            tc.tile_pool(name="xcp", bufs=4) as xcpool,
            tc.tile_pool(name="xe", bufs=2) as xepool,
            tc.tile_pool(name="xh", bufs=2) as xhpool,
            tc.tile_pool(name="h2o", bufs=2) as h2opool,
            tc.tile_pool(name="tmp", bufs=1) as tmp,
            tc.tile_pool(name="outp", bufs=2) as outp,
            tc.tile_pool(name="stat", bufs=1) as stat,
            tc.tile_pool(name="big", bufs=4, space="PSUM") as big,
            tc.tile_pool(name="pss", bufs=2, space="PSUM") as pss,
            tc.tile_pool(name="psy", bufs=2, space="PSUM") as psy,
        ):
            # ring assignment:
            #   A/B (sync + scalar HWDGE): xc and the big adj streams only
            #   G (gpsimd SWDGE): constants, bounces, collective triggers,
            #     gather pulls, output stores.  NOTE: keep collective-output
            #     reads off the sync ring (test_sync_dma_collective_hang).
            ringA, ringB = nc.sync, nc.scalar
            ringG = nc.gpsimd

            # --- replicated constants ---
            w1s = const.tile([128, 2, D], f8, tag="w1s")
            ringG.dma_start(w1s[:], w1.ap().rearrange("(a p) m -> p a m", p=128))
            w2s = const.tile([128, D], f16, tag="w2s")
            ringG.dma_start(w2s[:], w2.ap())
            wouts = const.tile([128, C], f32, tag="wouts")
            ringG.dma_start(wouts[:], wout.ap())
            bbs = const.tile([128, C], f32, tag="bbs")
            ringG.dma_start(bbs[:], bb.ap())
            corr1s = const.tile([128, 1], f32, tag="corr1s")
            ringG.dma_start(corr1s[:], corr1.ap())
            # fp32 copy of W2f for the (fp32) corr2 matvec
            w2s32 = const.tile([128, D], f32, tag="w2s32")
            nc.vector.tensor_copy(w2s32[:], w2s[:])

            # --- tiny warm-up collective: absorbs the CC-stream wakeup
            # latency (~11us) while the stream is otherwise idle ---
            zs = const.tile([1, 64], f8, tag="zs")
            nc.vector.memset(zs[:], 0.0)
            dumb = dram.tile([1, 64], f8, tag="dumb")
            dumf = dram.tile([ncores, 64], f8, tag="dumf", addr_space="Shared")
            ringG.dma_start(dumb[:], zs[:])
            ringG.collective_compute(
                "AllGather", OP.bypass, ins=[dumb.opt()], outs=[dumf.opt()],
                replica_groups=rg,
            )

            # --- DRAM bounce buffers for the collectives (partition-major) ---
            h2bA = dram.tile([128, pw], f8, tag="h2bA")
            h2bB = dram.tile([128, pw], f8, tag="h2bB")
            h2fA = dram.tile([ncores * 128, pw], f8, tag="h2fA", addr_space="Shared")
            h2fB = dram.tile([ncores * 128, pw], f8, tag="h2fB", addr_space="Shared")

            # ---- h1 = x @ W1f, replicated (DoubleRow over the two F-halves) ----
            # xc group g holds 8 chunk-lhsTs contiguous per partition:
            # xc[g*128+p, ((j*2+a)*128)+m] = xperm.T[a*128+p, (g*8+j)*128+m]
            hs1 = hs1p.tile([128, kb, D], f8, tag="hs1")
            xr = xc.ap().rearrange("(g p) q -> g p q", p=128)
            xg = None
            for k4 in range(kb // 4):
                ph = big.tile([128, 4, D], f32, tag="big", name=f"ph1_{k4}")
                for i in range(4):
                    k = k4 * 4 + i
                    g, j = divmod(k, 8)
                    if j == 0:
                        xg = xcpool.tile([128, 8, 2, 128], f8, tag="xg")
                        (ringA if g % 2 == 0 else ringB).dma_start(
                            xg.rearrange("p j a m -> p (j a m)"), xr[g]
                        )
                    nc.tensor.matmul(
                        ph[:, i, :], xg[:, j, :, :], w1s[:],
                        start=(i == 0), stop=(i == 3), perf_mode=DR,
                    )
                nc.vector.tensor_copy(hs1[:, k4 * 4 : (k4 + 1) * 4, :], ph[:])

            ar0 = adjt0.ap().rearrange("(g p j) m -> g p (j m)", p=128, j=kg)
            ar1 = adjt1.ap().rearrange("(g p j) m -> g p (j m)", p=128, j=kg)

            def stream_pass(ar, apool, nch, inject):
                """One reversed-order DoubleRow streaming pass over `ar`.

                nch: list of (psum_tile, o, w) output chunks.
                inject: {group_iteration_index: callback} runs after that
                group's matmuls are emitted.
                Returns {g: sbuf tile}; entries g < adj_bufs stay resident.
                """
                tiles = {}
                kgp = kg // 2
                for gi, g in enumerate(reversed(range(ngrp))):
                    at = apool.tile([128, kg * half], f8, tag="a")
                    hw_ = kg * half // 2
                    ringA.dma_start(at[:, 0:hw_], ar[g][:, 0:hw_])
                    ringB.dma_start(at[:, hw_:], ar[g][:, hw_:])
                    tiles[g] = at
                    atv = at.rearrange("p (j m) -> p j m", j=kg)
                    for jp in range(kgp):
                        kpi = gi * kgp + jp
                        kp = g * kg + 2 * jp  # first k-block of the pair
                        for ps, o, w in nch:
                            nc.tensor.matmul(
                                ps[:],
                                hs1[:, kp : kp + 2, :],
                                atv[:, 2 * jp : 2 * jp + 2, o : o + w],
                                start=(kpi == 0),
                                stop=(kpi == kb // 2 - 1),
                                perf_mode=DR,
                            )
                    if gi in inject:
                        inject[gi]()
                return tiles

            def elu_z(ps, o, w, dst, corrap, i):
                # dst[:, o:o+w] = elu(ps/16 + corr), fp32
                z = tmp.tile([128, 512], f32, tag="z", name=f"z{i}")
                nc.vector.tensor_scalar(
                    z[:, :w], ps[:], 1.0 / 16.0, corrap, op0=OP.mult, op1=OP.add
                )
                neg = tmp.tile([128, 512], f32, tag="neg", name=f"neg{i}")
                nc.vector.tensor_scalar_min(neg[:, :w], z[:, :w], 0.0)
                ex = tmp.tile([128, 512], f32, tag="ex", name=f"ex{i}")
                nc.scalar.activation(ex[:, :w], neg[:, :w], AF.Exp)
                pm1 = tmp.tile([128, 512], f32, tag="pm1", name=f"pm1{i}")
                nc.vector.tensor_scalar(
                    pm1[:, :w], z[:, :w], 0.0, -1.0, op0=OP.max, op1=OP.add
                )
                nc.vector.tensor_add(dst[:, o : o + w], ex[:, :w], pm1[:, :w])

            x2t = xepool.tile([128, s], f32, tag="xe")
            cs2 = stat.tile([128, 2], f32, tag="cs2")
            hs2 = hs2p.tile([128, kb, D], f8, tag="hs2")
            x2hs = {}
            cshl = {}

            def half_soft(hx, psn):
                # elu + colsum partial (hi/lo e4m3) + fp16 cast, half hx
                def emit():
                    for i, (ps, o, w) in enumerate(psn):
                        elu_z(ps, hx * half + o, w, x2t, corr1s[:, 0:1], i)
                    nc.vector.tensor_reduce(
                        cs2[:, hx : hx + 1],
                        x2t[:, hx * half : (hx + 1) * half],
                        axis=mybir.AxisListType.X,
                        op=OP.add,
                    )
                    x2h = xhpool.tile([128, half], f16, tag="xh", name=f"xh{hx}")
                    nc.vector.tensor_copy(x2h[:], x2t[:, hx * half : (hx + 1) * half])
                    x2hs[hx] = x2h

                return emit

            def half_bounce(hx, h2b, h2f):
                # own-h2 (node-major) + single-DMA bounce + AllGather trigger
                def emit():
                    x2h = x2hs[hx]
                    h2o = h2opool.tile([128, pw], f8, tag="h2o", name=f"h2o{hx}")
                    for c in range(c8):
                        ph = pss.tile([128, D], f32, tag="pss", name=f"h2o_{hx}_{c}")
                        nc.tensor.matmul(
                            ph[:],
                            x2h[:, c * 128 : (c + 1) * 128],
                            w2s[:],
                            start=True,
                            stop=True,
                        )
                        nc.vector.tensor_copy(h2o[:, c * D : (c + 1) * D], ph[:])
                    # hi/lo e4m3 of colsum/2048 written into the payload tile
                    t = stat.tile([128, 1], f32, tag=f"cst{hx}")
                    nc.vector.tensor_scalar_mul(t[:], cs2[:, hx : hx + 1], 1.0 / 2048.0)
                    nc.vector.tensor_copy(h2o[:, c8 * D : c8 * D + 1], t[:])
                    hi32 = stat.tile([128, 1], f32, tag=f"hi32{hx}")
                    nc.vector.tensor_copy(hi32[:], h2o[:, c8 * D : c8 * D + 1])
                    lo = stat.tile([128, 1], f32, tag=f"lo{hx}")
                    nc.vector.tensor_sub(lo[:], t[:], hi32[:])
                    nc.vector.tensor_scalar_mul(
                        h2o[:, c8 * D + 1 : c8 * D + 2], lo[:], 16.0
                    )
                    ringG.dma_start(h2b[:], h2o[:])
                    ringG.collective_compute(
                        "AllGather",
                        OP.bypass,
                        ins=[h2b.opt()],
                        outs=[h2f.opt()],
                        replica_groups=rg,
                    )

                return emit

            def pulls(hx, h2f, slot0):
                for r in range(ncores):
                    ringG.dma_start(
                        hs2[:, slot0 + r * c8 : slot0 + (r + 1) * c8, :],
                        h2f[r * 128 : (r + 1) * 128, 0 : c8 * 128],
                    )

            # ---- layer 1, pass A (adj columns 0:half) ----
            psA = [
                (big.tile([128, w], f32, tag="big", name=f"pA{i}"), o, w)
                for i, (o, w) in enumerate(chunks(half))
            ]
            tilesA = stream_pass(ar0, ap0, psA, {})

            # ---- layer 1, pass B; half-A epilogue injected into its stream ----
            psB = [
                (big.tile([128, w], f32, tag="big", name=f"pB{i}"), o, w)
                for i, (o, w) in enumerate(chunks(half))
            ]
            tilesB = stream_pass(
                ar1, ap1, psB,
                {0: half_soft(0, psA),
                 min(1, ngrp - 1): half_bounce(0, h2bA, h2fA)},
            )

            # half-B elu must be emitted BEFORE layer 2 reuses psB's banks;
            # its gather goes out before the L2 stream so the PE-idle window
            # between the passes absorbs the h2o matmuls
            half_soft(1, psB)()
            half_bounce(1, h2bB, h2fB)()
            # pulls AFTER both triggers: gpsimd is in-order, so a pull waiting
            # on gather A must not sit ahead of gather B's bounce/trigger
            pulls(0, h2fA, 0)
            pulls(1, h2fB, ncores * c8)

            # ---- corr2 assembly (emitted later into the PE stream) ----
            def corr2_assemble():
                for hx, h2f in ((0, h2fA), (1, h2fB)):
                    hv = h2f.rearrange("(r p) q -> q p r", p=128)
                    for t in (0, 1):
                        ringG.dma_start(
                            parts[:, t, hx * ncores : (hx + 1) * ncores],
                            hv[c8 * 128 + t],
                        )
                partsf = stat.tile([128, 2, 2 * ncores], f32, tag="partsf")
                nc.vector.tensor_copy(partsf[:], parts[:])
                ch = stat.tile([128, 2], f32, tag="ch")
                nc.vector.tensor_reduce(
                    ch[:, 0:1], partsf[:, 0, :], axis=mybir.AxisListType.X, op=OP.add
                )
                nc.vector.tensor_reduce(
                    ch[:, 1:2], partsf[:, 1, :], axis=mybir.AxisListType.X, op=OP.add
                )
                # colsum(x2) = 2048*CH + 128*CL = 128*(16*CH + CL)
                csum = stat.tile([128, 1], f32, tag="csum")
                nc.vector.tensor_scalar(
                    csum[:], ch[:, 0:1], 16.0, ch[:, 1:2], op0=OP.mult, op1=OP.add
                )
                pcv = psy.tile([128, 1], f32, tag="psy", name="pcv")
                nc.tensor.matmul(pcv[:], w2s32[:], csum[:], start=True, stop=True)
                # corr2 = 0.5 * 128 * pcv
                nc.vector.tensor_scalar_mul(corr2s[:], pcv[:], 64.0)

            parts = stat.tile([128, 2, 2 * ncores], f8, tag="parts")
            corr2s = stat.tile([128, 1], f32, tag="corr2s")

            # ---- layer 2: two column-half DoubleRow passes; each pass's
            # tail (elu + out layer + log_softmax) overlaps the next stream --
            kgp = kg // 2

            def l2_pass(ar, apool, tiles, nch):
                for g in range(ngrp):
                    if g < adj_bufs:
                        at = tiles[g]
                    else:
                        hw_ = kg * half // 2
                        at = apool.tile([128, kg * half], f8, tag="a")
                        ringA.dma_start(at[:, 0:hw_], ar[g][:, 0:hw_])
                        ringB.dma_start(at[:, hw_:], ar[g][:, hw_:])
                    av = at.rearrange("p (j m) -> p j m", j=kg)
                    for jp in range(kgp):
                        kpi = g * kgp + jp
                        kp = g * kg + 2 * jp
                        for ps, o, w in nch:
                            nc.tensor.matmul(
                                ps[:],
                                hs2[:, kp : kp + 2, :],
                                av[:, 2 * jp : 2 * jp + 2, o : o + w],
                                start=(kpi == 0),
                                stop=(kpi == kb // 2 - 1),
                                perf_mode=DR,
                            )

            x3t = xepool.tile([128, s], f32, tag="xe")
            outr = out.ap().rearrange("(c p) m -> c p m", p=128)

            def tail(hx, nch):
                # stage-wave emission: minimize in-order engine stalls
                base = hx * half
                zs = []
                for i, (ps, o, w) in enumerate(nch):
                    z = tmp.tile([128, 512], f32, tag="z", name=f"tz{hx}_{i}")
                    nc.vector.tensor_scalar(
                        z[:, :w], ps[:], 1.0 / 16.0, corr2s[:, 0:1],
                        op0=OP.mult, op1=OP.add,
                    )
                    zs.append(z)
                negs = []
                for i, (ps, o, w) in enumerate(nch):
                    neg = tmp.tile([128, 512], f32, tag="neg", name=f"tn{hx}_{i}")
                    nc.vector.tensor_scalar_min(neg[:, :w], zs[i][:, :w], 0.0)
                    negs.append(neg)
                exs = []
                for i, (ps, o, w) in enumerate(nch):
                    ex = tmp.tile([128, 512], f32, tag="ex", name=f"te{hx}_{i}")
                    nc.scalar.activation(ex[:, :w], negs[i][:, :w], AF.Exp)
                    exs.append(ex)
                pms = []
                for i, (ps, o, w) in enumerate(nch):
                    pm1 = tmp.tile([128, 512], f32, tag="pm1", name=f"tp{hx}_{i}")
                    nc.vector.tensor_scalar(
                        pm1[:, :w], zs[i][:, :w], 0.0, -1.0, op0=OP.max, op1=OP.add
                    )
                    pms.append(pm1)
                for i, (ps, o, w) in enumerate(nch):
                    nc.vector.tensor_add(
                        x3t[:, base + o : base + o + w], exs[i][:, :w], pms[i][:, :w]
                    )
                crng = list(range(base // 128, (base + half) // 128))
                pyb = psy.tile([128, c8, C], f32, tag="psy", name=f"pyb{hx}")
                for idx, c in enumerate(crng):
                    nc.tensor.matmul(
                        pyb[:, idx, :], x3t[:, c * 128 : (c + 1) * 128], wouts[:],
                        start=(idx == 0), stop=(idx == c8 - 1),
                    )
                zb = outp.tile([128, c8, C], f32, tag="zb", name=f"zb{hx}")
                for idx, c in enumerate(crng):
                    nc.vector.tensor_add(zb[:, idx, :], pyb[:, idx, :], bbs[:])
                zf = zb.rearrange("p c m -> p (c m)")
                negb = tmp.tile([128, c8 * C], f32, tag="neg", name=f"tnb{hx}")
                nc.vector.tensor_scalar_min(negb[:], zf, 0.0)
                eb = tmp.tile([128, c8 * C], f32, tag="ex", name=f"teb{hx}")
                nc.scalar.activation(eb[:], negb[:], AF.Exp)
                pmb = tmp.tile([128, c8 * C], f32, tag="pm1", name=f"tpb{hx}")
                nc.vector.tensor_scalar(pmb[:], zf, 0.0, -1.0, op0=OP.max, op1=OP.add)
                zzb = outp.tile([128, c8, C], f32, tag="zzb", name=f"zzb{hx}")
                nc.vector.tensor_add(zzb.rearrange("p c m -> p (c m)"), eb[:], pmb[:])
                negm = stat.tile([128, c8], f32, tag="negm", name=f"negm{hx}")
                nc.vector.tensor_reduce(
                    negm[:], zzb[:], axis=mybir.AxisListType.X, op=OP.max, negate=True
                )
                ssum = stat.tile([128, c8], f32, tag="ssum", name=f"ssum{hx}")
                es = tmp.tile([128, c8 * C], f32, tag="z", name=f"tes{hx}")
                esv = es.rearrange("p (c m) -> p c m", m=C)
                for idx in range(c8):
                    nc.scalar.activation(
                        esv[:, idx, :],
                        zzb[:, idx, :],
                        AF.Exp,
                        bias=negm[:, idx : idx + 1],
                        accum_out=ssum[:, idx : idx + 1],
                    )
                lse = stat.tile([128, c8], f32, tag="lse", name=f"lse{hx}")
                nc.scalar.activation(lse[:], ssum[:], AF.Ln)
                osbs = outp.tile([128, c8, C], f32, tag="osb", name=f"osb{hx}")
                for idx, c in enumerate(crng):
                    nc.vector.tensor_scalar(
                        osbs[:, idx, :],
                        zzb[:, idx, :],
                        negm[:, idx : idx + 1],
                        lse[:, idx : idx + 1],
                        op0=OP.add,
                        op1=OP.subtract,
                    )
                for idx, c in enumerate(crng):
                    ringG.dma_start(outr[c], osbs[:, idx, :])

            # pass alpha: own nodes 0:half (adjT0), full k
            psLa = [
                (big.tile([128, w], f32, tag="big", name=f"pLa{i}"), o, w)
                for i, (o, w) in enumerate(chunks(half))
            ]
            l2_pass(ar0, ap0, tilesA, psLa)
            corr2_assemble()
            # pass beta: own nodes half:s (adjT1); alpha tail overlaps it
            psLb = [
                (big.tile([128, w], f32, tag="big", name=f"pLb{i}"), o, w)
                for i, (o, w) in enumerate(chunks(half))
            ]
            tail(0, psLa)
            l2_pass(ar1, ap1, tilesB, psLb)
            tail(1, psLb)

    nc.compile()
    return nc


def make_in_maps(x, adj, W1, W2, Wout, bout, ncores=NCORES, kg=8):
    import ml_dtypes

    f8np = ml_dtypes.float8_e4m3
    n_total = adj.shape[0]
    s = n_total // ncores
    half = s // 2
    kb = n_total // 128
    ngrp = kb // kg
    kb8 = kb // 8
    f, d = W1.shape[1], W1.shape[0] * W1.shape[2]

    # global contraction-row order: all ranks' first halves, then seconds
    perm = np.concatenate(
        [np.arange(r * s, r * s + half) for r in range(ncores)]
        + [np.arange(r * s + half, (r + 1) * s) for r in range(ncores)]
    )

    w1f32 = np.ascontiguousarray(W1.transpose(1, 0, 2).reshape(f, d).astype(np.float32))
    w1f = w1f32.astype(f8np)
    w2f = np.ascontiguousarray(W2.transpose(1, 0, 2).reshape(d, d).astype(np.float16))
    woutf = np.ascontiguousarray(Wout.astype(np.float32))
    bbf = np.ascontiguousarray(
        np.broadcast_to(bout.astype(np.float32), (128, Wout.shape[1]))
    )
    # corr1 = 0.5 * colsum(h1) = 0.5 * colsum(x) @ W1f
    corr1 = (0.5 * (x.astype(np.float32).sum(0) @ w1f32)).astype(np.float32)
    corr1 = np.ascontiguousarray(corr1.reshape(d, 1))

    # fp8 e4m3 residual of adj, scaled by 16
    r8 = ((adj.astype(np.float32) - 0.5) * 16.0).astype(f8np)

    # xc[g*128 + p, ((j*2 + a)*128) + m] = xperm.T[a*128 + p, (g*8 + j)*128 + m]
    x8 = x.astype(f8np)
    xtc = x8[perm].T  # [F, n_total]
    xcf = np.ascontiguousarray(
        xtc.reshape(2, 128, kb8, 8, 128)
        .transpose(2, 1, 3, 0, 4)
        .reshape(kb8 * 128, 8 * f)
    )

    def reorder(t):
        # dram row (g*kg*128 + p*kg + j) <- k-row (g*kg*128 + j*128 + p)
        return np.ascontiguousarray(
            t.reshape(ngrp, kg, 128, t.shape[1]).transpose(0, 2, 1, 3).reshape(t.shape)
        )

    in_maps = []
    for c in range(ncores):
        t0 = r8[c * s : c * s + half, :][:, perm].T  # [n_total, half]
        t1 = r8[c * s + half : (c + 1) * s, :][:, perm].T
        in_maps.append(
            {
                "adjt0": reorder(t0),
                "adjt1": reorder(t1),
                "xc": xcf,
                "w1": w1f,
                "w2": w2f,
                "wout": woutf,
                "bb": bbf,
                "corr1": corr1,
            }
        )
    return in_maps


def kernel(x, adj, W1, W2, Wout, bout):
    from concourse import bass_utils

    x = np.asarray(x)
    adj = np.asarray(adj)
    in_maps = make_in_maps(x, adj, np.asarray(W1), np.asarray(W2),
                           np.asarray(Wout), np.asarray(bout))
    if "nc" not in _nc_cache:
        _nc_cache["nc"] = build_gat_nc()
    res = bass_utils.run_bass_kernel_spmd(
        _nc_cache["nc"], in_maps, core_ids=list(range(NCORES))
    )
    return np.concatenate([r["out"] for r in res.results], axis=0).astype(np.float32)
